# revision 22
# baseline (speedup 1.0000x reference)
"""Trainium2 Bass kernel for nn_AverageDistanceLoss.

Math: for batch item n with class c, unit quats qp/qg and model points
pts = points[c] ([P,3]):
  d_diag[p] = |Rp pts_p - Rg pts_p|^2 = pts_p^T S pts_p,  S = (Rp-Rg)^T (Rp-Rg)
  d_sym[p]  = min_q |Rp pts_p - Rg pts_q|^2
            = n_p + min_q (n_q - 2 pts_p^T R pts_q),      R = Rp^T Rg
(rotations preserve norms, n_p = |pts_p|^2). The heavy work per symmetric
item is a [P,P] pairwise matmul + row-min; everything index/quaternion
sized (O(B*C), O(B*P)) is host-side prep for sharding.

Sharding: data-parallel over batch. Symmetric items are dealt round-robin
across the 8 cores and packed 4-per-128-partition-group so the K=5 bf16
matmuls use 4 concurrent PE row-groups (tile_position). The row-min of
each [128,1024] G tile is computed by a custom DVE op (registered at
runtime): ACT stages one PSUM bank to SBUF, then one VectorE pass reads
PSUM+SBUF in parallel (rd0/rd1), pair-mins them, and min-folds into the
accumulator -> [128,1] per tile. Non-symmetric items need only d_diag,
batched into a single K=97 block-diagonal fp32 matmul per core. Each
core emits per-point hinge values; the final scalar sum happens on host.
"""

import numpy as np

NUM_CLASSES = 22
MARGIN = 0.01
B = 128
P = 1024
NCORES = 8
DIAG_SLOTS = 16  # max non-sym items per core (128/8)
DIAG_K = 6 * DIAG_SLOTS + 1  # 6 quadratic-form rows per item + one margin row

_RUNNER_CACHE: dict = {}


def _quat_to_rotmat(q):
    # q: [..., 4] (w, x, y, z) -> [..., 3, 3], float64
    w, x, y, z = q[..., 0], q[..., 1], q[..., 2], q[..., 3]
    r = np.empty(q.shape[:-1] + (3, 3), dtype=np.float64)
    r[..., 0, 0] = 1 - 2 * (y * y + z * z)
    r[..., 0, 1] = 2 * (x * y - z * w)
    r[..., 0, 2] = 2 * (x * z + y * w)
    r[..., 1, 0] = 2 * (x * y + z * w)
    r[..., 1, 1] = 1 - 2 * (x * x + z * z)
    r[..., 1, 2] = 2 * (y * z - x * w)
    r[..., 2, 0] = 2 * (x * z - y * w)
    r[..., 2, 1] = 2 * (y * z + x * w)
    r[..., 2, 2] = 1 - 2 * (x * x + y * y)
    return r


def _register_seg_min():
    """Author a custom DVE op: segmented pair-min-reduce. in0/in1 are
    [128, S, N]; for each segment s the op computes min over n of
    min(in0[:, s, n], in1[:, s, n]) in one streaming pass (rd0 = PSUM,
    rd1 = SBUF). A scan-min accumulator is re-seeded at each SUB_DIM_DONE
    boundary by an extra FSM state, and the destination AP's inner dim has
    step 0, so the running min overwrites one slot per segment — the last
    write is the segment min. No accumulator-readout companion op needed."""
    import copy

    import concourse.dve_ops as dve_ops_mod
    from concourse.dve_ops import DveOp
    from concourse.dve_spec import (
        AluOp,
        C0,
        Spec,
        Src0,
        Src1,
        minn,
        scan,
        lower as dve_lower,
    )
    from concourse.dve_uop import AluInp, DveOpSpec, Trigger

    name = "SEG_MIN_ANT"
    if name in dve_ops_mod._SUB_OPCODE_FOR_NAME:
        return next(op for op in dve_ops_mod.OPS if op.name == name)

    base_spec = Spec(body=scan(AluOp.MIN, minn(Src0, Src1), init=C0))

    def build_uops(ver):
        uops = dve_lower(base_spec, ver=ver)  # [seed, steady]
        assert len(uops) == 2
        seed, steady = uops[0], copy.deepcopy(uops[1])
        steady.trigger = (Trigger.SRC_TENSOR_DONE, Trigger.SUB_DIM_DONE, Trigger.NONE)
        steady.next_uop = (0, 2, 0)
        reseed = copy.deepcopy(steady)
        reseed.trigger = (Trigger.SRC_TENSOR_DONE, Trigger.SUB_DIM_DONE, Trigger.COUNT)
        reseed.next_uop = (0, 2, 1)
        reseed.repeat_count = 1
        dp1 = reseed.datapath_config[1]
        assert dp1.op == AluOp.MIN and dp1.alu_src0 == AluInp.CURR_ALU_OUT
        dp1.alu_src0 = AluInp.PREV_DELAY_2  # re-seed scan from the C0 lane
        return [seed, steady, reseed]

    class _RawUopDveOp(DveOp):
        def __post_init__(self):
            pass

        def compile(self, ver):
            key = (self.name, ver)
            cached = dve_ops_mod._COMPILE_CACHE.get(key)
            if cached is not None:
                return cached
            result = DveOpSpec(
                name=self.name,
                opcode=dve_ops_mod.get_dve_sub_opcode(self.name),
                uops=build_uops(ver),
                rd1_en=True,
            )
            assert self.uops_sha.get(ver) == result.sha(ver)
            dve_ops_mod._COMPILE_CACHE[key] = result
            return result

    row = dve_ops_mod._CUSTOM_DVE_ROW_BASE + len(dve_ops_mod.OPS)
    assert row < 0x20
    dve_ops_mod._SUB_OPCODE_FOR_NAME[name] = row
    shas = {}
    for ver in ("v3", "v4"):
        uops = build_uops(ver)
        shas[ver] = DveOpSpec(name=name, opcode=row, uops=uops, rd1_en=True).sha(ver)
    op = _RawUopDveOp(name, base_spec, subdim=True, uops_sha=shas)
    dve_ops_mod.OPS.append(op)
    dve_ops_mod.CUSTOM_DVE_SPECS[name] = base_spec
    return op


def _get_runner(n_slots: int, use_diag: bool):
    key = (n_slots, use_diag)
    if key in _RUNNER_CACHE:
        return _RUNNER_CACHE[key]

    import concourse.bass as bass
    import concourse.tile as tile
    from concourse import bacc, mybir
    from concourse.bass_utils import run_bass_kernel_spmd

    f32 = mybir.dt.float32
    bf16 = mybir.dt.bfloat16
    G = (n_slots + 3) // 4

    nc = bacc.Bacc("TRN2", target_bir_lowering=False, debug=False)

    ins = {}
    outs = {}
    if G > 0:
        ins["lhs"] = nc.dram_tensor("lhs", [G, 128, P], bf16, kind="ExternalInput").ap()
        ins["rhs"] = nc.dram_tensor("rhs", [G, 128, P], bf16, kind="ExternalInput").ap()
        outs["osym"] = nc.dram_tensor(
            "osym", [G, 128, 32], f32, kind="ExternalOutput"
        ).ap()
    if use_diag:
        ins["rhsd"] = nc.dram_tensor(
            "rhsd", [DIAG_K, P], f32, kind="ExternalInput"
        ).ap()
        ins["lhsd"] = nc.dram_tensor(
            "lhsd", [DIAG_K, DIAG_SLOTS], f32, kind="ExternalInput"
        ).ap()
        outs["odiag"] = nc.dram_tensor(
            "odiag", [DIAG_SLOTS, 1], f32, kind="ExternalOutput"
        ).ap()

    with tile.TileContext(nc) as tc:
        with (
            tc.tile_pool(name="big", bufs=2) as big,
            tc.tile_pool(name="route", bufs=5) as route,
            tc.tile_pool(name="small", bufs=3) as small,
            tc.tile_pool(name="psum", bufs=2, space=bass.MemorySpace.PSUM) as psum,
        ):
            if use_diag:
                rd = small.tile([DIAG_K, P], f32, tag="rd")
                nc.sync.dma_start(rd[:], ins["rhsd"][:])
                ld = small.tile([DIAG_K, DIAG_SLOTS], f32, tag="ld")
                nc.sync.dma_start(ld[:], ins["lhsd"][:])
            MINOP = _register_seg_min()
            biginit = small.tile([128, 1], f32, tag="biginit")
            nc.vector.memset(biginit[:], 3.0e38)
            for g in range(G):
                L = big.tile([128, P], bf16, tag="L")
                nc.sync.dma_start(L[:], ins["lhs"][g][:])
                R = big.tile([128, P], bf16, tag="R")
                nc.sync.dma_start(R[:], ins["rhs"][g][:])
                mins = small.tile([128, 32], f32, tag="mins")
                active = [s for s in range(4) if 4 * g + s < n_slots]
                if len(active) < 4:
                    nc.vector.memset(mins[:], 0.0)
                for m2 in range(4):
                    for s in active:
                        # Two 2-bank PSUM tiles per m-pair: j0 halves feed the
                        # DVE op directly; j1 halves are staged to SBUF by ACT
                        # (and their PSUM frees as soon as the copy is done).
                        pgj0 = psum.tile([128, 2, 512], f32, tag="pgA")
                        pgj1 = psum.tile([128, 2, 512], f32, tag="pgB")
                        for t in range(2):
                            m = 2 * m2 + t
                            lT = L[32 * s : 32 * s + 5, 128 * m : 128 * (m + 1)]
                            for pt, j in ((pgj1, 1), (pgj0, 0)):
                                nc.tensor.matmul(
                                    pt[:, t, :],
                                    lT,
                                    R[32 * s : 32 * s + 5, 512 * j : 512 * (j + 1)],
                                    start=True,
                                    stop=True,
                                    tile_position=(32 * s, 0),
                                )
                        c2 = route.tile([128, 2, 512], f32, tag="cp")
                        nc.scalar.copy(c2[:], pgj1[:])
                        col0 = 8 * s + 2 * m2
                        nc.vector._custom_dve(
                            MINOP,
                            out=mins[:, col0 : col0 + 2].to_broadcast([128, 2, 512]),
                            in0=pgj0[:],
                            in1=c2[:],
                            s0=biginit[:],
                        )
                hs = small.tile([128, 32], f32, tag="hs")
                nc.scalar.activation(
                    hs[:], mins[:], mybir.ActivationFunctionType.Relu
                )
                nc.sync.dma_start(outs["osym"][g][:], hs[:])

            if use_diag:
                pd = psum.tile([DIAG_SLOTS, P], f32, tag="pgA")
                for j in range(2):
                    nc.tensor.matmul(
                        pd[:, 512 * j : 512 * (j + 1)],
                        ld[:],
                        rd[:, 512 * j : 512 * (j + 1)],
                        start=True,
                        stop=True,
                    )
                hd = small.tile([DIAG_SLOTS, P], f32, tag="hd")
                dsum = small.tile([DIAG_SLOTS, 1], f32, tag="dsum")
                nc.scalar.activation(
                    hd[:],
                    pd[:],
                    mybir.ActivationFunctionType.Relu,
                    accum_out=dsum[:],
                )
                nc.sync.dma_start(outs["odiag"][:], dsum[:])
    nc.compile()

    def run(in_maps):
        res = run_bass_kernel_spmd(nc, in_maps, list(range(NCORES)))
        return res.results

    runner = (run, nc)
    _RUNNER_CACHE[key] = runner
    return runner


def _prepare(poses_pred, poses_target, poses_weight, points, symmetry):
    """Host-side shard prep. Returns (in_maps, n_groups, use_diag)."""
    poses_pred = np.asarray(poses_pred, dtype=np.float32)
    poses_target = np.asarray(poses_target, dtype=np.float32)
    poses_weight = np.asarray(poses_weight, dtype=np.float32)
    points = np.asarray(points, dtype=np.float32)
    symmetry = np.asarray(symmetry, dtype=np.float32)

    w = poses_weight.reshape(B, NUM_CLASSES, 4)[:, :, 0]
    has = w > 0
    valid = has.any(axis=1)
    cls = np.argmax(has, axis=1)
    sym = symmetry[cls] > 0

    rows = np.arange(B)
    qp = poses_pred.reshape(B, NUM_CLASSES, 4)[rows, cls].astype(np.float64)
    qg = poses_target.reshape(B, NUM_CLASSES, 4)[rows, cls].astype(np.float64)
    Rp = _quat_to_rotmat(qp)
    Rg = _quat_to_rotmat(qg)

    sym_items = [n for n in range(B) if valid[n] and sym[n]]
    diag_items = [n for n in range(B) if valid[n] and not sym[n]]

    S_max = max((len(sym_items[k::NCORES]) for k in range(NCORES)), default=0)
    G = (S_max + 3) // 4
    use_diag = len(diag_items) > 0

    # per-class precompute
    pts_f64 = points.astype(np.float64)  # [C, P, 3]
    nrm = (pts_f64**2).sum(-1)  # [C, P]

    in_maps = []
    for k in range(NCORES):
        im = {}
        if G > 0:
            import ml_dtypes
            lhs = np.zeros((G, 128, P), dtype=ml_dtypes.bfloat16)
            rhs = np.zeros((G, 128, P), dtype=ml_dtypes.bfloat16)
            for si, n in enumerate(sym_items[k::NCORES]):
                g, s = si // 4, si % 4
                c = cls[n]
                R = Rp[n].T @ Rg[n]
                z = pts_f64[c] @ R.T  # [P, 3]
                base = 32 * s
                lhs[g, base : base + 3, :] = -2.0 * pts_f64[c].T
                lhs[g, base + 3, :] = 1.0
                lhs[g, base + 4, :] = nrm[c] - MARGIN
                rhs[g, base : base + 3, :] = z.T
                rhs[g, base + 3, :] = nrm[c]
                rhs[g, base + 4, :] = 1.0
            im["lhs"] = lhs
            im["rhs"] = rhs
        if use_diag:
            rhsd = np.zeros((DIAG_K, P), dtype=np.float32)
            lhsd = np.zeros((DIAG_K, DIAG_SLOTS), dtype=np.float32)
            rhsd[6 * DIAG_SLOTS, :] = 1.0
            for d, n in enumerate(diag_items[k::NCORES]):
                c = cls[n]
                Sm = (Rp[n] - Rg[n]).T @ (Rp[n] - Rg[n])
                p3 = pts_f64[c]  # [P, 3]
                r = 6 * d
                rhsd[r + 0, :] = p3[:, 0] ** 2
                rhsd[r + 1, :] = p3[:, 1] ** 2
                rhsd[r + 2, :] = p3[:, 2] ** 2
                rhsd[r + 3, :] = p3[:, 0] * p3[:, 1]
                rhsd[r + 4, :] = p3[:, 0] * p3[:, 2]
                rhsd[r + 5, :] = p3[:, 1] * p3[:, 2]
                lhsd[r + 0, d] = Sm[0, 0]
                lhsd[r + 1, d] = Sm[1, 1]
                lhsd[r + 2, d] = Sm[2, 2]
                lhsd[r + 3, d] = 2.0 * Sm[0, 1]
                lhsd[r + 4, d] = 2.0 * Sm[0, 2]
                lhsd[r + 5, d] = 2.0 * Sm[1, 2]
                lhsd[6 * DIAG_SLOTS, d] = -MARGIN
            im["rhsd"] = rhsd
            im["lhsd"] = lhsd
        in_maps.append(im)
    return in_maps, S_max, use_diag


def kernel(poses_pred, poses_target, poses_weight, points, symmetry):
    in_maps, S_max, use_diag = _prepare(
        poses_pred, poses_target, poses_weight, points, symmetry
    )
    if S_max == 0 and not use_diag:
        return np.float32(0.0)

    run, _nc = _get_runner(S_max, use_diag)
    results = run(in_maps)

    G = (S_max + 3) // 4
    total = 0.0
    for k in range(NCORES):
        if G > 0:
            total += results[k]["osym"].astype(np.float64).sum()
        if use_diag:
            total += results[k]["odiag"].astype(np.float64).sum()
    return np.float32(0.5 * total / (B * P))


# revision 23
# speedup vs baseline: 1.0550x; 1.0550x over previous
"""Trainium2 Bass kernel for nn_AverageDistanceLoss.

Math: for batch item n with class c, unit quats qp/qg and model points
pts = points[c] ([P,3]):
  d_diag[p] = |Rp pts_p - Rg pts_p|^2 = pts_p^T S pts_p,  S = (Rp-Rg)^T (Rp-Rg)
  d_sym[p]  = min_q |Rp pts_p - Rg pts_q|^2
            = n_p + min_q (n_q - 2 pts_p^T R pts_q),      R = Rp^T Rg
(rotations preserve norms, n_p = |pts_p|^2). The heavy work per symmetric
item is a [P,P] pairwise matmul + row-min; everything index/quaternion
sized (O(B*C), O(B*P)) is host-side prep for sharding.

Sharding: data-parallel over batch. Symmetric items are dealt round-robin
across the 8 cores and packed 4-per-128-partition-group so the K=5 bf16
matmuls use 4 concurrent PE row-groups (tile_position). The row-min of
each [128,1024] G tile is computed by a custom DVE op (registered at
runtime): ACT stages one PSUM bank to SBUF, then one VectorE pass reads
PSUM+SBUF in parallel (rd0/rd1), pair-mins them, and min-folds into the
accumulator -> [128,1] per tile. Non-symmetric items need only d_diag,
batched into a single K=97 block-diagonal fp32 matmul per core. Each
core emits per-point hinge values; the final scalar sum happens on host.
"""

import numpy as np

NUM_CLASSES = 22
MARGIN = 0.01
B = 128
P = 1024
NCORES = 8
DIAG_SLOTS = 16  # max non-sym items per core (128/8)
DIAG_K = 6 * DIAG_SLOTS + 1  # 6 quadratic-form rows per item + one margin row

_RUNNER_CACHE: dict = {}


def _quat_to_rotmat(q):
    # q: [..., 4] (w, x, y, z) -> [..., 3, 3], float64
    w, x, y, z = q[..., 0], q[..., 1], q[..., 2], q[..., 3]
    r = np.empty(q.shape[:-1] + (3, 3), dtype=np.float64)
    r[..., 0, 0] = 1 - 2 * (y * y + z * z)
    r[..., 0, 1] = 2 * (x * y - z * w)
    r[..., 0, 2] = 2 * (x * z + y * w)
    r[..., 1, 0] = 2 * (x * y + z * w)
    r[..., 1, 1] = 1 - 2 * (x * x + z * z)
    r[..., 1, 2] = 2 * (y * z - x * w)
    r[..., 2, 0] = 2 * (x * z - y * w)
    r[..., 2, 1] = 2 * (y * z + x * w)
    r[..., 2, 2] = 1 - 2 * (x * x + y * y)
    return r


def _register_seg_min():
    """Author a custom DVE op: segmented pair-min-reduce. in0/in1 are
    [128, S, N]; for each segment s the op computes min over n of
    min(in0[:, s, n], in1[:, s, n]) in one streaming pass (rd0 = PSUM,
    rd1 = SBUF). A scan-min accumulator is re-seeded at each SUB_DIM_DONE
    boundary by an extra FSM state, and the destination AP's inner dim has
    step 0, so the running min overwrites one slot per segment — the last
    write is the segment min. No accumulator-readout companion op needed."""
    import copy

    import concourse.dve_ops as dve_ops_mod
    from concourse.dve_ops import DveOp
    from concourse.dve_spec import (
        AluOp,
        C0,
        Spec,
        Src0,
        Src1,
        minn,
        scan,
        lower as dve_lower,
    )
    from concourse.dve_uop import AluInp, DveOpSpec, Trigger

    name = "SEG_MIN_ANT"
    if name in dve_ops_mod._SUB_OPCODE_FOR_NAME:
        return next(op for op in dve_ops_mod.OPS if op.name == name)

    base_spec = Spec(body=scan(AluOp.MIN, minn(Src0, Src1), init=C0))

    def build_uops(ver):
        uops = dve_lower(base_spec, ver=ver)  # [seed, steady]
        assert len(uops) == 2
        seed, steady = uops[0], copy.deepcopy(uops[1])
        steady.trigger = (Trigger.SRC_TENSOR_DONE, Trigger.SUB_DIM_DONE, Trigger.NONE)
        steady.next_uop = (0, 2, 0)
        reseed = copy.deepcopy(steady)
        reseed.trigger = (Trigger.SRC_TENSOR_DONE, Trigger.SUB_DIM_DONE, Trigger.COUNT)
        reseed.next_uop = (0, 2, 1)
        reseed.repeat_count = 1
        dp1 = reseed.datapath_config[1]
        assert dp1.op == AluOp.MIN and dp1.alu_src0 == AluInp.CURR_ALU_OUT
        dp1.alu_src0 = AluInp.PREV_DELAY_2  # re-seed scan from the C0 lane
        return [seed, steady, reseed]

    class _RawUopDveOp(DveOp):
        def __post_init__(self):
            pass

        def compile(self, ver):
            key = (self.name, ver)
            cached = dve_ops_mod._COMPILE_CACHE.get(key)
            if cached is not None:
                return cached
            result = DveOpSpec(
                name=self.name,
                opcode=dve_ops_mod.get_dve_sub_opcode(self.name),
                uops=build_uops(ver),
                rd1_en=True,
            )
            assert self.uops_sha.get(ver) == result.sha(ver)
            dve_ops_mod._COMPILE_CACHE[key] = result
            return result

    row = dve_ops_mod._CUSTOM_DVE_ROW_BASE + len(dve_ops_mod.OPS)
    assert row < 0x20
    dve_ops_mod._SUB_OPCODE_FOR_NAME[name] = row
    shas = {}
    for ver in ("v3", "v4"):
        uops = build_uops(ver)
        shas[ver] = DveOpSpec(name=name, opcode=row, uops=uops, rd1_en=True).sha(ver)
    op = _RawUopDveOp(name, base_spec, subdim=True, uops_sha=shas)
    dve_ops_mod.OPS.append(op)
    dve_ops_mod.CUSTOM_DVE_SPECS[name] = base_spec
    return op


def _get_runner(n_slots: int, use_diag: bool):
    key = (n_slots, use_diag)
    if key in _RUNNER_CACHE:
        return _RUNNER_CACHE[key]

    import concourse.bass as bass
    import concourse.tile as tile
    from concourse import bacc, mybir
    from concourse.bass_utils import run_bass_kernel_spmd

    f32 = mybir.dt.float32
    bf16 = mybir.dt.bfloat16
    G = (n_slots + 3) // 4

    nc = bacc.Bacc("TRN2", target_bir_lowering=False, debug=False)

    ins = {}
    outs = {}
    if G > 0:
        ins["lhs"] = nc.dram_tensor("lhs", [G, 128, P], bf16, kind="ExternalInput").ap()
        ins["rhs"] = nc.dram_tensor("rhs", [G, 128, P], bf16, kind="ExternalInput").ap()
        outs["osym"] = nc.dram_tensor(
            "osym", [G, 128, 32], f32, kind="ExternalOutput"
        ).ap()
    if use_diag:
        ins["rhsd"] = nc.dram_tensor(
            "rhsd", [DIAG_K, P], f32, kind="ExternalInput"
        ).ap()
        ins["lhsd"] = nc.dram_tensor(
            "lhsd", [DIAG_K, DIAG_SLOTS], f32, kind="ExternalInput"
        ).ap()
        outs["odiag"] = nc.dram_tensor(
            "odiag", [DIAG_SLOTS, 1], f32, kind="ExternalOutput"
        ).ap()

    with tile.TileContext(nc) as tc:
        with (
            tc.tile_pool(name="big", bufs=2) as big,
            tc.tile_pool(name="route", bufs=5) as route,
            tc.tile_pool(name="small", bufs=3) as small,
            tc.tile_pool(name="psum", bufs=2, space=bass.MemorySpace.PSUM) as psum,
        ):
            MINOP = _register_seg_min()
            biginit = small.tile([128, 1], f32, tag="biginit")
            nc.vector.memset(biginit[:], 3.0e38)
            for g in range(G):
                L = big.tile([128, P], bf16, tag="L")
                R = big.tile([128, P], bf16, tag="R")
                if g == 0:
                    # chunk the first loads so slot 0's matmuls start early
                    for s4 in range(4):
                        rows = slice(32 * s4, 32 * (s4 + 1))
                        nc.sync.dma_start(L[rows, :], ins["lhs"][g][rows, :])
                        nc.sync.dma_start(R[rows, :], ins["rhs"][g][rows, :])
                else:
                    nc.sync.dma_start(L[:], ins["lhs"][g][:])
                    nc.sync.dma_start(R[:], ins["rhs"][g][:])
                mins = small.tile([128, 32], f32, tag="mins")
                active = [s for s in range(4) if 4 * g + s < n_slots]
                if len(active) < 4:
                    nc.vector.memset(mins[:], 0.0)
                for m2 in range(4):
                    for s in active:
                        # Two 2-bank PSUM tiles per m-pair: j0 halves feed the
                        # DVE op directly; j1 halves are staged to SBUF by ACT
                        # (and their PSUM frees as soon as the copy is done).
                        pgj0 = psum.tile([128, 2, 512], f32, tag="pgA")
                        pgj1 = psum.tile([128, 2, 512], f32, tag="pgB")
                        for t in range(2):
                            m = 2 * m2 + t
                            lT = L[32 * s : 32 * s + 5, 128 * m : 128 * (m + 1)]
                            for pt, j in ((pgj1, 1), (pgj0, 0)):
                                nc.tensor.matmul(
                                    pt[:, t, :],
                                    lT,
                                    R[32 * s : 32 * s + 5, 512 * j : 512 * (j + 1)],
                                    start=True,
                                    stop=True,
                                    tile_position=(32 * s, 0),
                                )
                        c2 = route.tile([128, 2, 512], f32, tag="cp")
                        nc.scalar.copy(c2[:], pgj1[:])
                        col0 = 8 * s + 2 * m2
                        nc.vector._custom_dve(
                            MINOP,
                            out=mins[:, col0 : col0 + 2].to_broadcast([128, 2, 512]),
                            in0=pgj0[:],
                            in1=c2[:],
                            s0=biginit[:],
                        )
                hs = small.tile([128, 32], f32, tag="hs")
                nc.scalar.activation(
                    hs[:], mins[:], mybir.ActivationFunctionType.Relu
                )
                nc.sync.dma_start(outs["osym"][g][:], hs[:])

            if use_diag:
                rd = small.tile([DIAG_K, P], f32, tag="rd")
                nc.sync.dma_start(rd[:], ins["rhsd"][:])
                ld = small.tile([DIAG_K, DIAG_SLOTS], f32, tag="ld")
                nc.sync.dma_start(ld[:], ins["lhsd"][:])
                pd = psum.tile([DIAG_SLOTS, P], f32, tag="pgA")
                for j in range(2):
                    nc.tensor.matmul(
                        pd[:, 512 * j : 512 * (j + 1)],
                        ld[:],
                        rd[:, 512 * j : 512 * (j + 1)],
                        start=True,
                        stop=True,
                    )
                hd = small.tile([DIAG_SLOTS, P], f32, tag="hd")
                dsum = small.tile([DIAG_SLOTS, 1], f32, tag="dsum")
                nc.scalar.activation(
                    hd[:],
                    pd[:],
                    mybir.ActivationFunctionType.Relu,
                    accum_out=dsum[:],
                )
                nc.sync.dma_start(outs["odiag"][:], dsum[:])
    nc.compile()

    def run(in_maps):
        res = run_bass_kernel_spmd(nc, in_maps, list(range(NCORES)))
        return res.results

    runner = (run, nc)
    _RUNNER_CACHE[key] = runner
    return runner


def _prepare(poses_pred, poses_target, poses_weight, points, symmetry):
    """Host-side shard prep. Returns (in_maps, n_groups, use_diag)."""
    poses_pred = np.asarray(poses_pred, dtype=np.float32)
    poses_target = np.asarray(poses_target, dtype=np.float32)
    poses_weight = np.asarray(poses_weight, dtype=np.float32)
    points = np.asarray(points, dtype=np.float32)
    symmetry = np.asarray(symmetry, dtype=np.float32)

    w = poses_weight.reshape(B, NUM_CLASSES, 4)[:, :, 0]
    has = w > 0
    valid = has.any(axis=1)
    cls = np.argmax(has, axis=1)
    sym = symmetry[cls] > 0

    rows = np.arange(B)
    qp = poses_pred.reshape(B, NUM_CLASSES, 4)[rows, cls].astype(np.float64)
    qg = poses_target.reshape(B, NUM_CLASSES, 4)[rows, cls].astype(np.float64)
    Rp = _quat_to_rotmat(qp)
    Rg = _quat_to_rotmat(qg)

    sym_items = [n for n in range(B) if valid[n] and sym[n]]
    diag_items = [n for n in range(B) if valid[n] and not sym[n]]

    S_max = max((len(sym_items[k::NCORES]) for k in range(NCORES)), default=0)
    G = (S_max + 3) // 4
    use_diag = len(diag_items) > 0

    # per-class precompute
    pts_f64 = points.astype(np.float64)  # [C, P, 3]
    nrm = (pts_f64**2).sum(-1)  # [C, P]

    in_maps = []
    for k in range(NCORES):
        im = {}
        if G > 0:
            import ml_dtypes
            lhs = np.zeros((G, 128, P), dtype=ml_dtypes.bfloat16)
            rhs = np.zeros((G, 128, P), dtype=ml_dtypes.bfloat16)
            for si, n in enumerate(sym_items[k::NCORES]):
                g, s = si // 4, si % 4
                c = cls[n]
                R = Rp[n].T @ Rg[n]
                z = pts_f64[c] @ R.T  # [P, 3]
                base = 32 * s
                lhs[g, base : base + 3, :] = -2.0 * pts_f64[c].T
                lhs[g, base + 3, :] = 1.0
                lhs[g, base + 4, :] = nrm[c] - MARGIN
                rhs[g, base : base + 3, :] = z.T
                rhs[g, base + 3, :] = nrm[c]
                rhs[g, base + 4, :] = 1.0
            im["lhs"] = lhs
            im["rhs"] = rhs
        if use_diag:
            rhsd = np.zeros((DIAG_K, P), dtype=np.float32)
            lhsd = np.zeros((DIAG_K, DIAG_SLOTS), dtype=np.float32)
            rhsd[6 * DIAG_SLOTS, :] = 1.0
            for d, n in enumerate(diag_items[k::NCORES]):
                c = cls[n]
                Sm = (Rp[n] - Rg[n]).T @ (Rp[n] - Rg[n])
                p3 = pts_f64[c]  # [P, 3]
                r = 6 * d
                rhsd[r + 0, :] = p3[:, 0] ** 2
                rhsd[r + 1, :] = p3[:, 1] ** 2
                rhsd[r + 2, :] = p3[:, 2] ** 2
                rhsd[r + 3, :] = p3[:, 0] * p3[:, 1]
                rhsd[r + 4, :] = p3[:, 0] * p3[:, 2]
                rhsd[r + 5, :] = p3[:, 1] * p3[:, 2]
                lhsd[r + 0, d] = Sm[0, 0]
                lhsd[r + 1, d] = Sm[1, 1]
                lhsd[r + 2, d] = Sm[2, 2]
                lhsd[r + 3, d] = 2.0 * Sm[0, 1]
                lhsd[r + 4, d] = 2.0 * Sm[0, 2]
                lhsd[r + 5, d] = 2.0 * Sm[1, 2]
                lhsd[6 * DIAG_SLOTS, d] = -MARGIN
            im["rhsd"] = rhsd
            im["lhsd"] = lhsd
        in_maps.append(im)
    return in_maps, S_max, use_diag


def kernel(poses_pred, poses_target, poses_weight, points, symmetry):
    in_maps, S_max, use_diag = _prepare(
        poses_pred, poses_target, poses_weight, points, symmetry
    )
    if S_max == 0 and not use_diag:
        return np.float32(0.0)

    run, _nc = _get_runner(S_max, use_diag)
    results = run(in_maps)

    G = (S_max + 3) // 4
    total = 0.0
    for k in range(NCORES):
        if G > 0:
            total += results[k]["osym"].astype(np.float64).sum()
        if use_diag:
            total += results[k]["odiag"].astype(np.float64).sum()
    return np.float32(0.5 * total / (B * P))


# revision 24
# speedup vs baseline: 1.0627x; 1.0073x over previous
"""Trainium2 Bass kernel for nn_AverageDistanceLoss.

Math: for batch item n with class c, unit quats qp/qg and model points
pts = points[c] ([P,3]):
  d_diag[p] = |Rp pts_p - Rg pts_p|^2 = pts_p^T S pts_p,  S = (Rp-Rg)^T (Rp-Rg)
  d_sym[p]  = min_q |Rp pts_p - Rg pts_q|^2
            = n_p + min_q (n_q - 2 pts_p^T R pts_q),      R = Rp^T Rg
(rotations preserve norms, n_p = |pts_p|^2). The heavy work per symmetric
item is a [P,P] pairwise matmul + row-min; everything index/quaternion
sized (O(B*C), O(B*P)) is host-side prep for sharding.

Sharding: data-parallel over batch. Symmetric items are dealt round-robin
across the 8 cores and packed 4-per-128-partition-group so the K=5 bf16
matmuls use 4 concurrent PE row-groups (tile_position). The row-min of
each [128,1024] G tile is computed by a custom DVE op (registered at
runtime): ACT stages one PSUM bank to SBUF, then one VectorE pass reads
PSUM+SBUF in parallel (rd0/rd1), pair-mins them, and min-folds into the
accumulator -> [128,1] per tile. Non-symmetric items need only d_diag,
batched into a single K=97 block-diagonal fp32 matmul per core. Each
core emits per-point hinge values; the final scalar sum happens on host.
"""

import numpy as np

NUM_CLASSES = 22
MARGIN = 0.01
B = 128
P = 1024
NCORES = 8
DIAG_SLOTS = 16  # max non-sym items per core (128/8)
DIAG_K = 6 * DIAG_SLOTS + 1  # 6 quadratic-form rows per item + one margin row

_RUNNER_CACHE: dict = {}


def _quat_to_rotmat(q):
    # q: [..., 4] (w, x, y, z) -> [..., 3, 3], float64
    w, x, y, z = q[..., 0], q[..., 1], q[..., 2], q[..., 3]
    r = np.empty(q.shape[:-1] + (3, 3), dtype=np.float64)
    r[..., 0, 0] = 1 - 2 * (y * y + z * z)
    r[..., 0, 1] = 2 * (x * y - z * w)
    r[..., 0, 2] = 2 * (x * z + y * w)
    r[..., 1, 0] = 2 * (x * y + z * w)
    r[..., 1, 1] = 1 - 2 * (x * x + z * z)
    r[..., 1, 2] = 2 * (y * z - x * w)
    r[..., 2, 0] = 2 * (x * z - y * w)
    r[..., 2, 1] = 2 * (y * z + x * w)
    r[..., 2, 2] = 1 - 2 * (x * x + y * y)
    return r


def _register_seg_min():
    """Author a custom DVE op: segmented pair-min-reduce. in0/in1 are
    [128, S, N]; for each segment s the op computes min over n of
    min(in0[:, s, n], in1[:, s, n]) in one streaming pass (rd0 = PSUM,
    rd1 = SBUF). A scan-min accumulator is re-seeded at each SUB_DIM_DONE
    boundary by an extra FSM state, and the destination AP's inner dim has
    step 0, so the running min overwrites one slot per segment — the last
    write is the segment min. No accumulator-readout companion op needed."""
    import copy

    import concourse.dve_ops as dve_ops_mod
    from concourse.dve_ops import DveOp
    from concourse.dve_spec import (
        AluOp,
        C0,
        Spec,
        Src0,
        Src1,
        minn,
        scan,
        lower as dve_lower,
    )
    from concourse.dve_uop import AluInp, DveOpSpec, Trigger

    name = "SEG_MIN_ANT"
    if name in dve_ops_mod._SUB_OPCODE_FOR_NAME:
        return next(op for op in dve_ops_mod.OPS if op.name == name)

    base_spec = Spec(body=scan(AluOp.MIN, minn(Src0, Src1), init=C0))

    def build_uops(ver):
        uops = dve_lower(base_spec, ver=ver)  # [seed, steady]
        assert len(uops) == 2
        seed, steady = uops[0], copy.deepcopy(uops[1])
        steady.trigger = (Trigger.SRC_TENSOR_DONE, Trigger.SUB_DIM_DONE, Trigger.NONE)
        steady.next_uop = (0, 2, 0)
        reseed = copy.deepcopy(steady)
        reseed.trigger = (Trigger.SRC_TENSOR_DONE, Trigger.SUB_DIM_DONE, Trigger.COUNT)
        reseed.next_uop = (0, 2, 1)
        reseed.repeat_count = 1
        dp1 = reseed.datapath_config[1]
        assert dp1.op == AluOp.MIN and dp1.alu_src0 == AluInp.CURR_ALU_OUT
        dp1.alu_src0 = AluInp.PREV_DELAY_2  # re-seed scan from the C0 lane
        return [seed, steady, reseed]

    class _RawUopDveOp(DveOp):
        def __post_init__(self):
            pass

        def compile(self, ver):
            key = (self.name, ver)
            cached = dve_ops_mod._COMPILE_CACHE.get(key)
            if cached is not None:
                return cached
            result = DveOpSpec(
                name=self.name,
                opcode=dve_ops_mod.get_dve_sub_opcode(self.name),
                uops=build_uops(ver),
                rd1_en=True,
            )
            assert self.uops_sha.get(ver) == result.sha(ver)
            dve_ops_mod._COMPILE_CACHE[key] = result
            return result

    row = dve_ops_mod._CUSTOM_DVE_ROW_BASE + len(dve_ops_mod.OPS)
    assert row < 0x20
    dve_ops_mod._SUB_OPCODE_FOR_NAME[name] = row
    shas = {}
    for ver in ("v3", "v4"):
        uops = build_uops(ver)
        shas[ver] = DveOpSpec(name=name, opcode=row, uops=uops, rd1_en=True).sha(ver)
    op = _RawUopDveOp(name, base_spec, subdim=True, uops_sha=shas)
    dve_ops_mod.OPS.append(op)
    dve_ops_mod.CUSTOM_DVE_SPECS[name] = base_spec
    return op


def _get_runner(n_slots: int, use_diag: bool):
    key = (n_slots, use_diag)
    if key in _RUNNER_CACHE:
        return _RUNNER_CACHE[key]

    import concourse.bass as bass
    import concourse.tile as tile
    from concourse import bacc, mybir
    from concourse.bass_utils import run_bass_kernel_spmd

    f32 = mybir.dt.float32
    bf16 = mybir.dt.bfloat16
    G = (n_slots + 3) // 4

    nc = bacc.Bacc("TRN2", target_bir_lowering=False, debug=False)

    ins = {}
    outs = {}
    if G > 0:
        ins["lhs"] = nc.dram_tensor("lhs", [G, 128, P], bf16, kind="ExternalInput").ap()
        ins["rhs"] = nc.dram_tensor("rhs", [G, 128, P], bf16, kind="ExternalInput").ap()
        outs["osym"] = nc.dram_tensor(
            "osym", [G, 128, 32], f32, kind="ExternalOutput"
        ).ap()
    if use_diag:
        ins["rhsd"] = nc.dram_tensor(
            "rhsd", [DIAG_K, P], f32, kind="ExternalInput"
        ).ap()
        ins["lhsd"] = nc.dram_tensor(
            "lhsd", [DIAG_K, DIAG_SLOTS], f32, kind="ExternalInput"
        ).ap()
        outs["odiag"] = nc.dram_tensor(
            "odiag", [DIAG_SLOTS, 1], f32, kind="ExternalOutput"
        ).ap()

    with tile.TileContext(nc) as tc:
        with (
            tc.tile_pool(name="big", bufs=2) as big,
            tc.tile_pool(name="route", bufs=5) as route,
            tc.tile_pool(name="small", bufs=3) as small,
            tc.tile_pool(name="psum", bufs=2, space=bass.MemorySpace.PSUM) as psum,
        ):
            MINOP = _register_seg_min()
            biginit = small.tile([128, 1], f32, tag="biginit")
            nc.vector.memset(biginit[:], 3.0e38)
            gorder = list(range(G))
            if n_slots % 4:
                gorder = [G - 1] + gorder[:-1]  # partial group first
            for gi, g in enumerate(gorder):
                L = big.tile([128, P], bf16, tag="L")
                R = big.tile([128, P], bf16, tag="R")
                if gi == 0:
                    # chunk the first loads so slot 0's matmuls start early
                    for s4 in range(4):
                        rows = slice(32 * s4, 32 * (s4 + 1))
                        nc.sync.dma_start(L[rows, :], ins["lhs"][g][rows, :])
                        nc.sync.dma_start(R[rows, :], ins["rhs"][g][rows, :])
                else:
                    nc.sync.dma_start(L[:], ins["lhs"][g][:])
                    nc.sync.dma_start(R[:], ins["rhs"][g][:])
                mins = small.tile([128, 32], f32, tag="mins")
                active = [s for s in range(4) if 4 * g + s < n_slots]
                if len(active) < 4:
                    nc.vector.memset(mins[:], 0.0)
                for m2 in range(4):
                    for s in active:
                        # Two 2-bank PSUM tiles per m-pair: j0 halves feed the
                        # DVE op directly; j1 halves are staged to SBUF by ACT
                        # (and their PSUM frees as soon as the copy is done).
                        pgj0 = psum.tile([128, 2, 512], f32, tag="pgA")
                        pgj1 = psum.tile([128, 2, 512], f32, tag="pgB")
                        for t in range(2):
                            m = 2 * m2 + t
                            lT = L[32 * s : 32 * s + 5, 128 * m : 128 * (m + 1)]
                            for pt, j in ((pgj1, 1), (pgj0, 0)):
                                nc.tensor.matmul(
                                    pt[:, t, :],
                                    lT,
                                    R[32 * s : 32 * s + 5, 512 * j : 512 * (j + 1)],
                                    start=True,
                                    stop=True,
                                    tile_position=(32 * s, 0),
                                )
                        c2 = route.tile([128, 2, 512], f32, tag="cp")
                        nc.scalar.copy(c2[:], pgj1[:])
                        col0 = 8 * s + 2 * m2
                        nc.vector._custom_dve(
                            MINOP,
                            out=mins[:, col0 : col0 + 2].to_broadcast([128, 2, 512]),
                            in0=pgj0[:],
                            in1=c2[:],
                            s0=biginit[:],
                        )
                hs = small.tile([128, 32], f32, tag="hs")
                nc.scalar.activation(
                    hs[:], mins[:], mybir.ActivationFunctionType.Relu
                )
                nc.sync.dma_start(outs["osym"][g][:], hs[:])

            if use_diag:
                rd = small.tile([DIAG_K, P], f32, tag="rd")
                nc.sync.dma_start(rd[:], ins["rhsd"][:])
                ld = small.tile([DIAG_K, DIAG_SLOTS], f32, tag="ld")
                nc.sync.dma_start(ld[:], ins["lhsd"][:])
                pd = psum.tile([DIAG_SLOTS, P], f32, tag="pgA")
                for j in range(2):
                    nc.tensor.matmul(
                        pd[:, 512 * j : 512 * (j + 1)],
                        ld[:],
                        rd[:, 512 * j : 512 * (j + 1)],
                        start=True,
                        stop=True,
                    )
                hd = small.tile([DIAG_SLOTS, P], f32, tag="hd")
                dsum = small.tile([DIAG_SLOTS, 1], f32, tag="dsum")
                nc.scalar.activation(
                    hd[:],
                    pd[:],
                    mybir.ActivationFunctionType.Relu,
                    accum_out=dsum[:],
                )
                nc.sync.dma_start(outs["odiag"][:], dsum[:])
    nc.compile()

    def run(in_maps):
        res = run_bass_kernel_spmd(nc, in_maps, list(range(NCORES)))
        return res.results

    runner = (run, nc)
    _RUNNER_CACHE[key] = runner
    return runner


def _prepare(poses_pred, poses_target, poses_weight, points, symmetry):
    """Host-side shard prep. Returns (in_maps, n_groups, use_diag)."""
    poses_pred = np.asarray(poses_pred, dtype=np.float32)
    poses_target = np.asarray(poses_target, dtype=np.float32)
    poses_weight = np.asarray(poses_weight, dtype=np.float32)
    points = np.asarray(points, dtype=np.float32)
    symmetry = np.asarray(symmetry, dtype=np.float32)

    w = poses_weight.reshape(B, NUM_CLASSES, 4)[:, :, 0]
    has = w > 0
    valid = has.any(axis=1)
    cls = np.argmax(has, axis=1)
    sym = symmetry[cls] > 0

    rows = np.arange(B)
    qp = poses_pred.reshape(B, NUM_CLASSES, 4)[rows, cls].astype(np.float64)
    qg = poses_target.reshape(B, NUM_CLASSES, 4)[rows, cls].astype(np.float64)
    Rp = _quat_to_rotmat(qp)
    Rg = _quat_to_rotmat(qg)

    sym_items = [n for n in range(B) if valid[n] and sym[n]]
    diag_items = [n for n in range(B) if valid[n] and not sym[n]]

    S_max = max((len(sym_items[k::NCORES]) for k in range(NCORES)), default=0)
    G = (S_max + 3) // 4
    use_diag = len(diag_items) > 0

    # per-class precompute
    pts_f64 = points.astype(np.float64)  # [C, P, 3]
    nrm = (pts_f64**2).sum(-1)  # [C, P]

    in_maps = []
    for k in range(NCORES):
        im = {}
        if G > 0:
            import ml_dtypes
            lhs = np.zeros((G, 128, P), dtype=ml_dtypes.bfloat16)
            rhs = np.zeros((G, 128, P), dtype=ml_dtypes.bfloat16)
            for si, n in enumerate(sym_items[k::NCORES]):
                g, s = si // 4, si % 4
                c = cls[n]
                R = Rp[n].T @ Rg[n]
                z = pts_f64[c] @ R.T  # [P, 3]
                base = 32 * s
                lhs[g, base : base + 3, :] = -2.0 * pts_f64[c].T
                lhs[g, base + 3, :] = 1.0
                lhs[g, base + 4, :] = nrm[c] - MARGIN
                rhs[g, base : base + 3, :] = z.T
                rhs[g, base + 3, :] = nrm[c]
                rhs[g, base + 4, :] = 1.0
            im["lhs"] = lhs
            im["rhs"] = rhs
        if use_diag:
            rhsd = np.zeros((DIAG_K, P), dtype=np.float32)
            lhsd = np.zeros((DIAG_K, DIAG_SLOTS), dtype=np.float32)
            rhsd[6 * DIAG_SLOTS, :] = 1.0
            for d, n in enumerate(diag_items[k::NCORES]):
                c = cls[n]
                Sm = (Rp[n] - Rg[n]).T @ (Rp[n] - Rg[n])
                p3 = pts_f64[c]  # [P, 3]
                r = 6 * d
                rhsd[r + 0, :] = p3[:, 0] ** 2
                rhsd[r + 1, :] = p3[:, 1] ** 2
                rhsd[r + 2, :] = p3[:, 2] ** 2
                rhsd[r + 3, :] = p3[:, 0] * p3[:, 1]
                rhsd[r + 4, :] = p3[:, 0] * p3[:, 2]
                rhsd[r + 5, :] = p3[:, 1] * p3[:, 2]
                lhsd[r + 0, d] = Sm[0, 0]
                lhsd[r + 1, d] = Sm[1, 1]
                lhsd[r + 2, d] = Sm[2, 2]
                lhsd[r + 3, d] = 2.0 * Sm[0, 1]
                lhsd[r + 4, d] = 2.0 * Sm[0, 2]
                lhsd[r + 5, d] = 2.0 * Sm[1, 2]
                lhsd[6 * DIAG_SLOTS, d] = -MARGIN
            im["rhsd"] = rhsd
            im["lhsd"] = lhsd
        in_maps.append(im)
    return in_maps, S_max, use_diag


def kernel(poses_pred, poses_target, poses_weight, points, symmetry):
    in_maps, S_max, use_diag = _prepare(
        poses_pred, poses_target, poses_weight, points, symmetry
    )
    if S_max == 0 and not use_diag:
        return np.float32(0.0)

    run, _nc = _get_runner(S_max, use_diag)
    results = run(in_maps)

    G = (S_max + 3) // 4
    total = 0.0
    for k in range(NCORES):
        if G > 0:
            total += results[k]["osym"].astype(np.float64).sum()
        if use_diag:
            total += results[k]["odiag"].astype(np.float64).sum()
    return np.float32(0.5 * total / (B * P))


# revision 25
# speedup vs baseline: 1.0636x; 1.0009x over previous
"""Trainium2 Bass kernel for nn_AverageDistanceLoss.

Math: for batch item n with class c, unit quats qp/qg and model points
pts = points[c] ([P,3]):
  d_diag[p] = |Rp pts_p - Rg pts_p|^2 = pts_p^T S pts_p,  S = (Rp-Rg)^T (Rp-Rg)
  d_sym[p]  = min_q |Rp pts_p - Rg pts_q|^2
            = n_p + min_q (n_q - 2 pts_p^T R pts_q),      R = Rp^T Rg
(rotations preserve norms, n_p = |pts_p|^2). The heavy work per symmetric
item is a [P,P] pairwise matmul + row-min; everything index/quaternion
sized (O(B*C), O(B*P)) is host-side prep for sharding.

Sharding: data-parallel over batch. Symmetric items are dealt round-robin
across the 8 cores and packed 4-per-128-partition-group so the K=5 bf16
matmuls use 4 concurrent PE row-groups (tile_position). The row-min of
each [128,1024] G tile is computed by a custom DVE op (registered at
runtime): ACT stages one PSUM bank to SBUF, then one VectorE pass reads
PSUM+SBUF in parallel (rd0/rd1), pair-mins them, and min-folds into the
accumulator -> [128,1] per tile. Non-symmetric items need only d_diag,
batched into a single K=97 block-diagonal fp32 matmul per core. Each
core emits per-point hinge values; the final scalar sum happens on host.
"""

import numpy as np

NUM_CLASSES = 22
MARGIN = 0.01
B = 128
P = 1024
NCORES = 8
DIAG_SLOTS = 16  # max non-sym items per core (128/8)
DIAG_K = 6 * DIAG_SLOTS + 1  # 6 quadratic-form rows per item + one margin row

_RUNNER_CACHE: dict = {}


def _quat_to_rotmat(q):
    # q: [..., 4] (w, x, y, z) -> [..., 3, 3], float64
    w, x, y, z = q[..., 0], q[..., 1], q[..., 2], q[..., 3]
    r = np.empty(q.shape[:-1] + (3, 3), dtype=np.float64)
    r[..., 0, 0] = 1 - 2 * (y * y + z * z)
    r[..., 0, 1] = 2 * (x * y - z * w)
    r[..., 0, 2] = 2 * (x * z + y * w)
    r[..., 1, 0] = 2 * (x * y + z * w)
    r[..., 1, 1] = 1 - 2 * (x * x + z * z)
    r[..., 1, 2] = 2 * (y * z - x * w)
    r[..., 2, 0] = 2 * (x * z - y * w)
    r[..., 2, 1] = 2 * (y * z + x * w)
    r[..., 2, 2] = 1 - 2 * (x * x + y * y)
    return r


def _register_seg_min():
    """Author a custom DVE op: segmented pair-min-reduce. in0/in1 are
    [128, S, N]; for each segment s the op computes min over n of
    min(in0[:, s, n], in1[:, s, n]) in one streaming pass (rd0 = PSUM,
    rd1 = SBUF). A scan-min accumulator is re-seeded at each SUB_DIM_DONE
    boundary by an extra FSM state, and the destination AP's inner dim has
    step 0, so the running min overwrites one slot per segment — the last
    write is the segment min. No accumulator-readout companion op needed."""
    import copy

    import concourse.dve_ops as dve_ops_mod
    from concourse.dve_ops import DveOp
    from concourse.dve_spec import (
        AluOp,
        C0,
        Spec,
        Src0,
        Src1,
        minn,
        scan,
        lower as dve_lower,
    )
    from concourse.dve_uop import AluInp, DveOpSpec, Trigger

    name = "SEG_MIN_ANT"
    if name in dve_ops_mod._SUB_OPCODE_FOR_NAME:
        return next(op for op in dve_ops_mod.OPS if op.name == name)

    base_spec = Spec(body=scan(AluOp.MIN, minn(Src0, Src1), init=C0))

    def build_uops(ver):
        uops = dve_lower(base_spec, ver=ver)  # [seed, steady]
        assert len(uops) == 2
        seed, steady = uops[0], copy.deepcopy(uops[1])
        steady.trigger = (Trigger.SRC_TENSOR_DONE, Trigger.SUB_DIM_DONE, Trigger.NONE)
        steady.next_uop = (0, 2, 0)
        reseed = copy.deepcopy(steady)
        reseed.trigger = (Trigger.SRC_TENSOR_DONE, Trigger.SUB_DIM_DONE, Trigger.COUNT)
        reseed.next_uop = (0, 2, 1)
        reseed.repeat_count = 1
        dp1 = reseed.datapath_config[1]
        assert dp1.op == AluOp.MIN and dp1.alu_src0 == AluInp.CURR_ALU_OUT
        dp1.alu_src0 = AluInp.PREV_DELAY_2  # re-seed scan from the C0 lane
        return [seed, steady, reseed]

    class _RawUopDveOp(DveOp):
        def __post_init__(self):
            pass

        def compile(self, ver):
            key = (self.name, ver)
            cached = dve_ops_mod._COMPILE_CACHE.get(key)
            if cached is not None:
                return cached
            result = DveOpSpec(
                name=self.name,
                opcode=dve_ops_mod.get_dve_sub_opcode(self.name),
                uops=build_uops(ver),
                rd1_en=True,
            )
            assert self.uops_sha.get(ver) == result.sha(ver)
            dve_ops_mod._COMPILE_CACHE[key] = result
            return result

    row = dve_ops_mod._CUSTOM_DVE_ROW_BASE + len(dve_ops_mod.OPS)
    assert row < 0x20
    dve_ops_mod._SUB_OPCODE_FOR_NAME[name] = row
    shas = {}
    for ver in ("v3", "v4"):
        uops = build_uops(ver)
        shas[ver] = DveOpSpec(name=name, opcode=row, uops=uops, rd1_en=True).sha(ver)
    op = _RawUopDveOp(name, base_spec, subdim=True, uops_sha=shas)
    dve_ops_mod.OPS.append(op)
    dve_ops_mod.CUSTOM_DVE_SPECS[name] = base_spec
    return op


def _get_runner(n_slots: int, use_diag: bool):
    key = (n_slots, use_diag)
    if key in _RUNNER_CACHE:
        return _RUNNER_CACHE[key]

    import concourse.bass as bass
    import concourse.tile as tile
    from concourse import bacc, mybir
    from concourse.bass_utils import run_bass_kernel_spmd

    f32 = mybir.dt.float32
    bf16 = mybir.dt.bfloat16
    G = (n_slots + 3) // 4

    nc = bacc.Bacc("TRN2", target_bir_lowering=False, debug=False)

    ins = {}
    outs = {}
    if G > 0:
        ins["lhs"] = nc.dram_tensor("lhs", [G, 128, P], bf16, kind="ExternalInput").ap()
        ins["rhs"] = nc.dram_tensor("rhs", [G, 128, P], bf16, kind="ExternalInput").ap()
        outs["osym"] = nc.dram_tensor(
            "osym", [G, 128, 32], f32, kind="ExternalOutput"
        ).ap()
    if use_diag:
        ins["rhsd"] = nc.dram_tensor(
            "rhsd", [DIAG_K, P], f32, kind="ExternalInput"
        ).ap()
        ins["lhsd"] = nc.dram_tensor(
            "lhsd", [DIAG_K, DIAG_SLOTS], f32, kind="ExternalInput"
        ).ap()
        outs["odiag"] = nc.dram_tensor(
            "odiag", [DIAG_SLOTS, 1], f32, kind="ExternalOutput"
        ).ap()

    with tile.TileContext(nc) as tc:
        with (
            tc.tile_pool(name="big", bufs=2) as big,
            tc.tile_pool(name="route", bufs=5) as route,
            tc.tile_pool(name="small", bufs=3) as small,
            tc.tile_pool(name="psum", bufs=2, space=bass.MemorySpace.PSUM) as psum,
        ):
            MINOP = _register_seg_min()
            biginit = small.tile([128, 1], f32, tag="biginit")
            nc.vector.memset(biginit[:], 3.0e38)
            gorder = list(range(G))
            if n_slots % 4:
                gorder = [G - 1] + gorder[:-1]  # partial group first
            for gi, g in enumerate(gorder):
                active = [s for s in range(4) if 4 * g + s < n_slots]
                L = big.tile([128, P], bf16, tag="L")
                R = big.tile([128, P], bf16, tag="R")
                if gi == 0:
                    # chunk the first loads (active slots only) so the first
                    # matmuls start as early as possible
                    for s4 in active:
                        rows = slice(32 * s4, 32 * (s4 + 1))
                        nc.sync.dma_start(L[rows, :], ins["lhs"][g][rows, :])
                        nc.sync.dma_start(R[rows, :], ins["rhs"][g][rows, :])
                else:
                    nc.sync.dma_start(L[:], ins["lhs"][g][:])
                    nc.sync.dma_start(R[:], ins["rhs"][g][:])
                mins = small.tile([128, 32], f32, tag="mins")
                if len(active) < 4:
                    nc.vector.memset(mins[:], 0.0)
                for m2 in range(4):
                    for s in active:
                        # Two 2-bank PSUM tiles per m-pair: j0 halves feed the
                        # DVE op directly; j1 halves are staged to SBUF by ACT
                        # (and their PSUM frees as soon as the copy is done).
                        pgj0 = psum.tile([128, 2, 512], f32, tag="pgA")
                        pgj1 = psum.tile([128, 2, 512], f32, tag="pgB")
                        for t in range(2):
                            m = 2 * m2 + t
                            lT = L[32 * s : 32 * s + 5, 128 * m : 128 * (m + 1)]
                            for pt, j in ((pgj1, 1), (pgj0, 0)):
                                nc.tensor.matmul(
                                    pt[:, t, :],
                                    lT,
                                    R[32 * s : 32 * s + 5, 512 * j : 512 * (j + 1)],
                                    start=True,
                                    stop=True,
                                    tile_position=(32 * s, 0),
                                )
                        c2 = route.tile([128, 2, 512], f32, tag="cp")
                        nc.scalar.copy(c2[:], pgj1[:])
                        col0 = 8 * s + 2 * m2
                        nc.vector._custom_dve(
                            MINOP,
                            out=mins[:, col0 : col0 + 2].to_broadcast([128, 2, 512]),
                            in0=pgj0[:],
                            in1=c2[:],
                            s0=biginit[:],
                        )
                hs = small.tile([128, 32], f32, tag="hs")
                nc.scalar.activation(
                    hs[:], mins[:], mybir.ActivationFunctionType.Relu
                )
                nc.sync.dma_start(outs["osym"][g][:], hs[:])

            if use_diag:
                rd = small.tile([DIAG_K, P], f32, tag="rd")
                nc.sync.dma_start(rd[:], ins["rhsd"][:])
                ld = small.tile([DIAG_K, DIAG_SLOTS], f32, tag="ld")
                nc.sync.dma_start(ld[:], ins["lhsd"][:])
                pd = psum.tile([DIAG_SLOTS, P], f32, tag="pgA")
                for j in range(2):
                    nc.tensor.matmul(
                        pd[:, 512 * j : 512 * (j + 1)],
                        ld[:],
                        rd[:, 512 * j : 512 * (j + 1)],
                        start=True,
                        stop=True,
                    )
                hd = small.tile([DIAG_SLOTS, P], f32, tag="hd")
                dsum = small.tile([DIAG_SLOTS, 1], f32, tag="dsum")
                nc.scalar.activation(
                    hd[:],
                    pd[:],
                    mybir.ActivationFunctionType.Relu,
                    accum_out=dsum[:],
                )
                nc.sync.dma_start(outs["odiag"][:], dsum[:])
    nc.compile()

    def run(in_maps):
        res = run_bass_kernel_spmd(nc, in_maps, list(range(NCORES)))
        return res.results

    runner = (run, nc)
    _RUNNER_CACHE[key] = runner
    return runner


def _prepare(poses_pred, poses_target, poses_weight, points, symmetry):
    """Host-side shard prep. Returns (in_maps, n_groups, use_diag)."""
    poses_pred = np.asarray(poses_pred, dtype=np.float32)
    poses_target = np.asarray(poses_target, dtype=np.float32)
    poses_weight = np.asarray(poses_weight, dtype=np.float32)
    points = np.asarray(points, dtype=np.float32)
    symmetry = np.asarray(symmetry, dtype=np.float32)

    w = poses_weight.reshape(B, NUM_CLASSES, 4)[:, :, 0]
    has = w > 0
    valid = has.any(axis=1)
    cls = np.argmax(has, axis=1)
    sym = symmetry[cls] > 0

    rows = np.arange(B)
    qp = poses_pred.reshape(B, NUM_CLASSES, 4)[rows, cls].astype(np.float64)
    qg = poses_target.reshape(B, NUM_CLASSES, 4)[rows, cls].astype(np.float64)
    Rp = _quat_to_rotmat(qp)
    Rg = _quat_to_rotmat(qg)

    sym_items = [n for n in range(B) if valid[n] and sym[n]]
    diag_items = [n for n in range(B) if valid[n] and not sym[n]]

    S_max = max((len(sym_items[k::NCORES]) for k in range(NCORES)), default=0)
    G = (S_max + 3) // 4
    use_diag = len(diag_items) > 0

    # per-class precompute
    pts_f64 = points.astype(np.float64)  # [C, P, 3]
    nrm = (pts_f64**2).sum(-1)  # [C, P]

    in_maps = []
    for k in range(NCORES):
        im = {}
        if G > 0:
            import ml_dtypes
            lhs = np.zeros((G, 128, P), dtype=ml_dtypes.bfloat16)
            rhs = np.zeros((G, 128, P), dtype=ml_dtypes.bfloat16)
            for si, n in enumerate(sym_items[k::NCORES]):
                g, s = si // 4, si % 4
                c = cls[n]
                R = Rp[n].T @ Rg[n]
                z = pts_f64[c] @ R.T  # [P, 3]
                base = 32 * s
                lhs[g, base : base + 3, :] = -2.0 * pts_f64[c].T
                lhs[g, base + 3, :] = 1.0
                lhs[g, base + 4, :] = nrm[c] - MARGIN
                rhs[g, base : base + 3, :] = z.T
                rhs[g, base + 3, :] = nrm[c]
                rhs[g, base + 4, :] = 1.0
            im["lhs"] = lhs
            im["rhs"] = rhs
        if use_diag:
            rhsd = np.zeros((DIAG_K, P), dtype=np.float32)
            lhsd = np.zeros((DIAG_K, DIAG_SLOTS), dtype=np.float32)
            rhsd[6 * DIAG_SLOTS, :] = 1.0
            for d, n in enumerate(diag_items[k::NCORES]):
                c = cls[n]
                Sm = (Rp[n] - Rg[n]).T @ (Rp[n] - Rg[n])
                p3 = pts_f64[c]  # [P, 3]
                r = 6 * d
                rhsd[r + 0, :] = p3[:, 0] ** 2
                rhsd[r + 1, :] = p3[:, 1] ** 2
                rhsd[r + 2, :] = p3[:, 2] ** 2
                rhsd[r + 3, :] = p3[:, 0] * p3[:, 1]
                rhsd[r + 4, :] = p3[:, 0] * p3[:, 2]
                rhsd[r + 5, :] = p3[:, 1] * p3[:, 2]
                lhsd[r + 0, d] = Sm[0, 0]
                lhsd[r + 1, d] = Sm[1, 1]
                lhsd[r + 2, d] = Sm[2, 2]
                lhsd[r + 3, d] = 2.0 * Sm[0, 1]
                lhsd[r + 4, d] = 2.0 * Sm[0, 2]
                lhsd[r + 5, d] = 2.0 * Sm[1, 2]
                lhsd[6 * DIAG_SLOTS, d] = -MARGIN
            im["rhsd"] = rhsd
            im["lhsd"] = lhsd
        in_maps.append(im)
    return in_maps, S_max, use_diag


def kernel(poses_pred, poses_target, poses_weight, points, symmetry):
    in_maps, S_max, use_diag = _prepare(
        poses_pred, poses_target, poses_weight, points, symmetry
    )
    if S_max == 0 and not use_diag:
        return np.float32(0.0)

    run, _nc = _get_runner(S_max, use_diag)
    results = run(in_maps)

    G = (S_max + 3) // 4
    total = 0.0
    for k in range(NCORES):
        if G > 0:
            total += results[k]["osym"].astype(np.float64).sum()
        if use_diag:
            total += results[k]["odiag"].astype(np.float64).sum()
    return np.float32(0.5 * total / (B * P))
